# Initial kernel scaffold
#
"""Distributed Trainium2 kernel for a 2-relation GNN message-passing layer.

agg = x @ W_self.T + sum_r scatter_add(x[src_r] @ W_r.T, tgt_r)

Strategy (8 NeuronCores, SPMD, no collectives):
- Targets sharded: core c owns rows [c*62500, (c+1)*62500) of the output.
- x (bf16) replicated to every core as 16 bucket tensors of 31250 rows
  (dma_gather uses int16 indices, so a gather table must stay < 32768 rows).
- Phase A: per (quarter, relation, src-bucket), dma_gather the edges' source
  rows (bucket-local int16 idxs) and write them to a DRAM staging buffer in a
  known order.  Staging is ordered by (relation, tgt-quarter); each quarter
  holds < 32768 rows so it can itself be a gather table.  Within each chunk
  the edges are target-sorted, so phase-B windows see sorted targets.
- Phase B: per (512-target window, relation), dma_gather the window's edge
  rows from staging in target-sorted order, build narrow one-hot slices
  T[slot, t] on DVE (compare streamed target values against an iota row over
  only the block's active target range - compile-time known from the sorted
  packing, unioned across cores for SPMD) and matmul-accumulate
  gT_w[f, t] += G[slot, f] * T[slot, t] into a DVE-zeroed PSUM bank.  This is
  the scatter-add, done by TensorE with ~3x less DVE/PE work than a
  full-width one-hot.
- Per window: out_w[t, o] = xT_w.T @ Wself^T + g0T_w.T @ W0^T + g1T_w.T @ W1^T
  accumulated in a second PSUM bank, copied to bf16 and written to a
  partition-major output tensor (big DMA descriptors); the host reassembles
  and upcasts.

All instruction shapes are identical across cores (SPMD); per-core variation
lives in the index tensors.  Unused trailing slots gather row 0 (a valid row)
and their one-hot target value is -1 (matches no column -> contributes zero).
"""
import os
import sys
import types

import numpy as np

sys.path.insert(0, "/opt/trn_rl_repo")

N = 500_000
D = 128
NUM_REL = 2
NCORE = 8
TPC = N // NCORE            # 62500 targets per core
NB = 16                     # src buckets
BROWS = N // NB             # 31250 rows per bucket
WIN = 512                   # aggregation window (one PSUM bank: 512 f32)
NWIN = (TPC + WIN - 1) // WIN          # 123 windows (last = 36 targets)
QB = [0, 15360, 30720, 46080, 62500]   # window-aligned target quarters
NQ = 4
ACAPS_Q = [(1024, 128), (1024, 128), (1024, 128), (1024, 256)]
BCAP = 640                  # phase-B slots per (rel, window) chunk
NQUEUE = 4


def _register_profile_hook():
    if "antenv.axon_hooks" in sys.modules:
        return
    mod = types.ModuleType("antenv.axon_hooks")
    state = {"h": None}
    mod.set_axon_ntff_profile_hook = lambda h: state.__setitem__("h", h)
    mod.get_axon_ntff_profile_hook = lambda: state["h"]
    sys.modules["antenv.axon_hooks"] = mod
    try:
        from trn_agent_boot.trn_boot import _ntff_profile_via_ctypes
        mod.set_axon_ntff_profile_hook(
            _ntff_profile_via_ctypes("/opt/axon/libaxon_pjrt.so"))
    except Exception:
        pass


def _achunks():
    """Static phase-A chunk table: (rel, quarter, bucket, cap, staging_row_off).
    Chunks of one (rel, quarter) occupy consecutive rows of stage[rel][q]."""
    table = []
    rows_q = [0] * NQ
    for r in range(NUM_REL):
        for q in range(NQ):
            off = 0
            for b in range(NB):
                for cap in ACAPS_Q[q]:
                    table.append((r, q, b, cap, off))
                    off += cap
            rows_q[q] = off
            assert off <= 32767, off
    return table, rows_q


ACHUNKS, STAGE_ROWS_Q = _achunks()

# windows: (w, quarter, t0, tn)
WINDOWS = []
for w in range(NWIN):
    t0 = w * WIN
    tn = min(WIN, TPC - t0)
    q = max(i for i in range(NQ) if QB[i] <= t0)
    WINDOWS.append((w, q, t0, tn))


def _pack_core(edge_indices, core):
    """Build per-core index tensors.

    Returns:
      agidx [A_SLOTS] int16   phase-A gather idxs (bucket-local src), -1 pad
      bgidx [B_SLOTS] int16   phase-B gather idxs (staging row), -1 pad
      btv   [B_SLOTS] f32     phase-B target value local to window, -1 pad
    """
    lo = core * TPC
    # pads gather row 0 (valid); their one-hot target value is -1 -> zero
    agidx = np.zeros(sum(c[3] for c in ACHUNKS), dtype=np.int16)
    bgidx = np.zeros(len(WINDOWS) * NUM_REL * BCAP, dtype=np.int16)
    btv = np.full(len(WINDOWS) * NUM_REL * BCAP, -1.0, dtype=np.float32)
    avalid = np.zeros(len(ACHUNKS), dtype=np.int64)
    bvalid = np.zeros(len(WINDOWS) * NUM_REL, dtype=np.int64)

    for r in range(NUM_REL):
        src = np.asarray(edge_indices[r, 0])
        tgt = np.asarray(edge_indices[r, 1])
        m = (tgt >= lo) & (tgt < lo + TPC)
        s = src[m]
        t = (tgt[m] - lo).astype(np.int64)
        for h in range(NQ):
            hm = (t >= QB[h]) & (t < QB[h + 1])
            sh, th = s[hm], t[hm]
            order = np.argsort(th, kind="stable")
            sh, th = sh[order], th[order]
            b = sh // BROWS
            # staging row for each edge: chunks are bucket-major
            stage_row = np.empty(sh.shape[0], dtype=np.int64)
            # phase-A fill
            for bb in range(NB):
                bm = b == bb
                es = (sh[bm] - bb * BROWS).astype(np.int16)
                chunks = [(cap, soff, fb, ci) for ci, ((rr, hh, bbx, cap, soff), fb)
                          in enumerate(zip(ACHUNKS, _AFLAT))
                          if rr == r and hh == h and bbx == bb]
                n = es.shape[0]
                total_cap = sum(c[0] for c in chunks)
                if n > total_cap:
                    raise RuntimeError(f"phase-A overflow r{r} h{h} b{bb}: {n}")
                pos = 0
                rows = np.empty(n, dtype=np.int64)
                for cap, soff, fb, ci in chunks:
                    k = min(cap, n - pos)
                    if k > 0:
                        agidx[fb:fb + k] = es[pos:pos + k]
                        # stage rows are written permuted (partition-major:
                        # slot s lands at soff + (s%128)*nblk + s//128) so the
                        # staging DMA is one big contiguous descriptor per
                        # partition instead of 256B per row
                        sl = np.arange(k)
                        nblk = cap // 128
                        rows[pos:pos + k] = soff + (sl % 128) * nblk + sl // 128
                        pos += k
                    avalid[ci] = k
                stage_row[np.nonzero(bm)[0]] = rows
            # phase-B fill: windows over this quarter (edges stay target-sorted)
            for (w, wh, t0, tn) in WINDOWS:
                if wh != h:
                    continue
                wm = (th >= t0) & (th < t0 + tn)
                er = stage_row[wm]
                et = th[wm] - t0
                k = er.shape[0]
                if k > BCAP:
                    raise RuntimeError(f"phase-B overflow r{r} w{w}: {k}")
                boff = (w * NUM_REL + r) * BCAP
                bgidx[boff:boff + k] = er.astype(np.int16)
                btv[boff:boff + k] = et.astype(np.float32)
                bvalid[w * NUM_REL + r] = k
    return agidx, bgidx, btv, avalid, bvalid


# flat slot offsets for phase-A chunks (concatenated across (rel, quarter))
_AFLAT = []
_fb = 0
for (_r, _h, _b, _cap, _soff) in ACHUNKS:
    _AFLAT.append(_fb)
    _fb += _cap
A_SLOTS = _fb
B_SLOTS = len(WINDOWS) * NUM_REL * BCAP
OUTBLK = NWIN * 4           # 128-row col-blocks in the p-major output


def _wrap16(idx_flat):
    n = idx_flat.shape[0]
    a = idx_flat.reshape(n // 16, 16).T
    return np.tile(a, (8, 1)).copy()


def _slotmaj(v, width):
    """[n] -> [128, n//128 * width]-style slot-major layout [p, blk] where
    slot i -> [i%128, i//128] (gather output layout)."""
    n = v.shape[0]
    return np.ascontiguousarray(v.reshape(n // 128, 128).T)


def _block_ranges(packs, bregs):
    """Per (window, rel, 128-slot block): union across cores of the active
    window-local target range [lo, hi) covered by that block's edges.
    All-pad blocks get (0, 1)."""
    ranges = {}
    for (w, h, t0, tn) in WINDOWS:
        for r in range(NUM_REL):
            wi = w * NUM_REL + r
            boff = wi * BCAP
            nblk = (int(bregs[wi]) + 127) // 128
            for j in range(nblk):
                lo, hi = tn, 0
                for (agidx, bgidx, btv, av, bv) in packs:
                    blk = btv[boff + j * 128: boff + (j + 1) * 128]
                    blk = blk[blk >= 0]
                    if blk.size:
                        lo = min(lo, int(blk.min()))
                        hi = max(hi, int(blk.max()) + 1)
                if hi <= lo:
                    lo, hi = 0, 1
                ranges[(w, r, j)] = (lo, hi)
    return ranges


def _build_program(aregs, bregs, ranges):
    import concourse.bacc as bacc
    import concourse.tile as tile
    from concourse import mybir

    nc = bacc.Bacc("TRN2", debug=False, num_swdge_queues=NQUEUE)
    dt = mybir.dt

    xb = [nc.dram_tensor(f"xb{k}", [BROWS, D], dt.bfloat16, kind="ExternalInput")
          for k in range(NB)]
    xto = nc.dram_tensor("xto", [D, TPC], dt.bfloat16, kind="ExternalInput")
    wt = nc.dram_tensor("wt", [D, 3 * D], dt.bfloat16, kind="ExternalInput")
    agidx_d = nc.dram_tensor("agidx", [128, A_SLOTS // 16], dt.int16, kind="ExternalInput")
    bgidx_d = nc.dram_tensor("bgidx", [128, B_SLOTS // 16], dt.int16, kind="ExternalInput")
    btv_d = nc.dram_tensor("btv", [128, B_SLOTS // 128], dt.float16, kind="ExternalInput")
    iota_d = nc.dram_tensor("iota", [128, WIN], dt.float16, kind="ExternalInput")
    stage = [[nc.dram_tensor(f"stage{r}{h}", [STAGE_ROWS_Q[h], D], dt.bfloat16,
                             kind="ExternalOutput")
              for h in range(NQ)] for r in range(NUM_REL)]
    # partition-major output: target t = (blk*128 + p) lives at out[p, blk, :]
    out_d = nc.dram_tensor("out", [128, OUTBLK, D], dt.bfloat16,
                           kind="ExternalOutput")

    with tile.TileContext(nc) as tc:
        with (
            tc.tile_pool(name="const", bufs=1) as cpool,
            tc.tile_pool(name="ag", bufs=8) as agpool,
            tc.tile_pool(name="bg", bufs=4) as bgpool,
            tc.tile_pool(name="oh", bufs=10) as ohpool,
            tc.tile_pool(name="gsb", bufs=6) as gsbpool,
            tc.tile_pool(name="xt", bufs=4) as xtpool,
            tc.tile_pool(name="psA", bufs=4, space="PSUM") as psA,
            tc.tile_pool(name="psB", bufs=4, space="PSUM") as psB,
        ):
            wt_sb = cpool.tile([D, 3 * D], dt.bfloat16)
            nc.sync.dma_start(wt_sb[:], wt[:])
            iota_sb = cpool.tile([128, WIN], dt.float16)
            nc.sync.dma_start(iota_sb[:], iota_d[:])
            agidx_sb = cpool.tile([128, A_SLOTS // 16], dt.int16)
            nc.sync.dma_start(agidx_sb[:], agidx_d[:])
            bgidx_sb = cpool.tile([128, B_SLOTS // 16], dt.int16)
            nc.sync.dma_start(bgidx_sb[:], bgidx_d[:])
            btv_sb = cpool.tile([128, B_SLOTS // 128], dt.float16)
            nc.sync.dma_start(btv_sb[:], btv_d[:])

            # ---- phase A: gather x rows into staging ----
            # Quarter-major with relations interleaved so each quarter's
            # staging (both rels) completes early and phase B overlaps phase A.
            _aorder = sorted(range(len(ACHUNKS)),
                             key=lambda ci: (ACHUNKS[ci][1], ACHUNKS[ci][0],
                                             ACHUNKS[ci][2], -ACHUNKS[ci][3]))
            for ci in _aorder:
                (r, h, b, cap, soff) = ACHUNKS[ci]
                fb = _AFLAT[ci]
                g = agpool.tile([128, 10, D], dt.bfloat16, tag="ag")
                nblk = cap // 128
                nc.gpsimd.dma_gather(
                    g[:, :nblk, :], xb[b][:],
                    agidx_sb[:, fb // 16:(fb + cap) // 16],
                    cap, int(aregs[ci]), D, queue_num=ci % NQUEUE,
                )
                nc.sync.dma_start(
                    stage[r][h][soff:soff + cap, :].rearrange(
                        "(p j) o -> p j o", p=128),
                    g[:, :nblk, :],
                )

            # ---- phase B: per (window, rel) gather + narrow one-hot agg ----
            _maxwin = int(os.environ.get("KMAXWIN", "123"))
            _phase_a_only = os.environ.get("KPHASE", "") == "A"
            gb_bufs = []
            for i in range(10):
                t = cpool.tile([128, BCAP // 128, D], dt.bfloat16, tag=f"gbb{i}")
                nc.vector.memset(t[:], 0.0)
                gb_bufs.append(t)
            gb_rot = 0
            for (w, h, t0, tn) in WINDOWS:
                if _phase_a_only or w >= _maxwin:
                    continue
                outp = psB.tile([128, 4, D], dt.float32, tag="psB")
                # self term first
                xt_t = xtpool.tile([D, WIN], dt.bfloat16, tag="xt")
                nc.sync.dma_start(xt_t[:, :tn], xto[:, t0:t0 + tn])
                nsub_t = (tn + 127) // 128
                for j in range(nsub_t):
                    wdt = min(128, tn - j * 128)
                    nc.tensor.matmul(
                        outp[:wdt, j, :],
                        xt_t[:, j * 128:j * 128 + wdt],
                        wt_sb[:, 0:D],
                        start=(j == 0), stop=False,
                    )
                for r in range(NUM_REL):
                    boff = (w * NUM_REL + r) * BCAP
                    gb = gb_bufs[gb_rot % 10]
                    gb_rot += 1
                    nc.gpsimd.dma_gather(
                        gb[:], stage[r][h][:],
                        bgidx_sb[:, boff // 16:(boff + BCAP) // 16],
                        BCAP, int(bregs[w * NUM_REL + r]), D, transpose=False,
                        queue_num=(w * NUM_REL + r) % NQUEUE,
                    )
                    gps = psA.tile([128, WIN], dt.float32, tag="psA")
                    nc.vector.memset(gps[:, :tn], 0.0)
                    nblk_v = (int(bregs[w * NUM_REL + r]) + 127) // 128
                    for j in range(nblk_v):
                        lo, hi = ranges[(w, r, j)]
                        oh = ohpool.tile([128, WIN], dt.bfloat16, tag="oh")
                        nc.vector.tensor_tensor(
                            out=oh[:, :hi - lo],
                            in0=btv_sb[:, boff // 128 + j:boff // 128 + j + 1]
                                .to_broadcast([128, hi - lo]),
                            in1=iota_sb[:, lo:hi],
                            op=mybir.AluOpType.is_equal,
                        )
                        nc.tensor.matmul(
                            gps[:, lo:hi],
                            gb[:, j, :],
                            oh[:, :hi - lo],
                            start=False, stop=(j == nblk_v - 1),
                        )
                    gsb = gsbpool.tile([128, WIN], dt.bfloat16, tag="gsb")
                    nc.scalar.copy(out=gsb[:, :tn], in_=gps[:, :tn])
                    for j in range(nsub_t):
                        wdt = min(128, tn - j * 128)
                        nc.tensor.matmul(
                            outp[:wdt, j, :],
                            gsb[:, j * 128:j * 128 + wdt],
                            wt_sb[:, (1 + r) * D:(2 + r) * D],
                            start=False,
                            stop=(r == NUM_REL - 1 and j == nsub_t - 1),
                        )
                osb = xtpool.tile([128, 4, D], dt.bfloat16, tag="osb")
                nc.scalar.copy(out=osb[:, :nsub_t, :], in_=outp[:, :nsub_t, :])
                nc.sync.dma_start(
                    out_d[:, w * 4:w * 4 + nsub_t, :],
                    osb[:, :nsub_t, :],
                )
    nc.compile()
    return nc


_NC_CACHE = {}


def kernel(x, W0, W1, W_self, edge_indices):
    import ml_dtypes
    from concourse import bass_utils
    from concourse.bass_utils import run_bass_kernel_spmd

    _register_profile_hook()
    bass_utils.upload_artifacts = lambda tmpdir: "local://" + tmpdir

    x = np.asarray(x)
    W0 = np.asarray(W0)
    W1 = np.asarray(W1)
    W_self = np.asarray(W_self)
    edge_indices = np.asarray(edge_indices)

    bf16 = ml_dtypes.bfloat16
    x16 = x.astype(bf16)
    xbufs = [np.ascontiguousarray(x16[k * BROWS:(k + 1) * BROWS]) for k in range(NB)]
    wt = np.concatenate([W_self.T, W0.T, W1.T], axis=1).astype(bf16)
    iota = np.tile(np.arange(WIN, dtype=np.float16), (128, 1))

    packs = [_pack_core(edge_indices, c) for c in range(NCORE)]
    aregs = np.max([p[3] for p in packs], axis=0)
    bregs = np.max([p[4] for p in packs], axis=0)
    ranges = _block_ranges(packs, bregs)
    if "nc" not in _NC_CACHE:
        _NC_CACHE["nc"] = _build_program(aregs, bregs, ranges)
    nc = _NC_CACHE["nc"]

    in_maps = []
    for c in range(NCORE):
        agidx, bgidx, btv, av, bv = packs[c]
        # beyond the per-instruction register count, use -1 (ucode trims these
        # to exactly the register count on every core -> consistent bookkeeping)
        for ci, (r_, h_, b_, cap, soff) in enumerate(ACHUNKS):
            fb = _AFLAT[ci]
            agidx[fb + int(aregs[ci]):fb + cap] = -1
        for wi in range(len(WINDOWS) * NUM_REL):
            boff = wi * BCAP
            bgidx[boff + int(bregs[wi]):boff + BCAP] = -1
        im = {f"xb{k}": xbufs[k] for k in range(NB)}
        im["xto"] = np.ascontiguousarray(x16[c * TPC:(c + 1) * TPC].T)
        im["wt"] = wt
        im["agidx"] = _wrap16(agidx)
        im["bgidx"] = _wrap16(bgidx)
        im["btv"] = _slotmaj(btv.astype(np.float16), 1)
        im["iota"] = iota
        in_maps.append(im)

    trace = os.environ.get("KBENCH_TRACE", "0") == "1"
    res = run_bass_kernel_spmd(nc, in_maps, core_ids=list(range(NCORE)),
                               trace=trace)
    if trace:
        print("HW exec time:", res.exec_time_ns, "ns")
        _NC_CACHE["exec_time_ns"] = res.exec_time_ns

    out = np.empty((N, D), dtype=np.float32)
    for c in range(NCORE):
        o = np.asarray(res.results[c]["out"])          # [128, OUTBLK, 128] bf16
        o = o.transpose(1, 0, 2).reshape(OUTBLK * 128, D)[:TPC]
        out[c * TPC:(c + 1) * TPC] = o.astype(np.float32)
    return out



# revision 11
# speedup vs baseline: 2.0141x; 2.0141x over previous
"""Distributed Trainium2 kernel for a 2-relation GNN message-passing layer.

agg = x @ W_self.T + sum_r scatter_add(x[src_r] @ W_r.T, tgt_r)

Strategy (8 NeuronCores, SPMD, no collectives):
- Targets sharded: core c owns rows [c*62500, (c+1)*62500) of the output.
- x (bf16) replicated to every core as 16 bucket tensors of 31250 rows
  (dma_gather uses int16 indices, so gather tables stay < 32768 rows).
- Targets split into 8 regions of 16 windows (512 targets each).  Staging
  table per region holds BOTH relations (2 rel x 16 buckets x 640 rows =
  20480 < 32768), so one phase-B gather serves a window-pair across both
  relations.
- Phase A: per (rel, bucket, region-quad) one dma_gather of 4x640 = 2560
  bucket-local rows, written to the 4 region staging tables as big
  contiguous partition-major descriptors.  64 gathers total.
- Phase B: per window-pair one dma_gather of 2560 rows from the region
  staging table in target-sorted order (62 gathers).  Per window:
  one-hot scatter matmuls (5 blocks x narrow ranges per rel) into a
  PSUM agg bank gps[f, t], then transposed GEMMs
  out[o, t] = Wself.T @ xT + W0.T @ agg0 + W1.T @ agg1, each one matmul
  streaming 512 columns, accumulated in a second PSUM bank.
- All instruction shapes identical across cores (SPMD); per-core variation
  lives in the index tensors.  Pad slots gather row 0 (valid, finite) and
  carry one-hot target value -1 (matches nothing -> contributes zero).
"""
import os
import sys
import types

import numpy as np

sys.path.insert(0, "/opt/trn_rl_repo")

N = 500_000
D = 128
NUM_REL = 2
NCORE = 8
TPC = N // NCORE            # 62500 targets per core
NB = 16                     # src buckets
BROWS = N // NB             # 31250 rows per bucket
WIN = 512                   # aggregation window (one PSUM bank: 512 f32)
NWIN = (TPC + WIN - 1) // WIN          # 123 windows
WPR = 16                    # windows per region
NREG = (NWIN + WPR - 1) // WPR         # 8 regions
SUBCAP = 640                # slots per (rel, bucket, region) staging chunk
BCAP = 640                  # slots per (rel, window) in phase B
STAGE_ROWS = NUM_REL * NB * SUBCAP     # 20480 rows per region table
NPAIR = (NWIN + 1) // 2                # 62 window-pairs
PAIR_SLOTS = 4 * BCAP                  # 2560 slots per phase-B gather
A_GATHER_SLOTS = 4 * SUBCAP            # 2560 slots per phase-A gather
A_SLOTS = NUM_REL * NB * NREG * SUBCAP     # 163840
B_SLOTS = NPAIR * PAIR_SLOTS               # 158720
TPCP = NWIN * WIN           # 62976 padded target count
NQUEUE = 4


def _register_profile_hook():
    if "antenv.axon_hooks" in sys.modules:
        return
    mod = types.ModuleType("antenv.axon_hooks")
    state = {"h": None}
    mod.set_axon_ntff_profile_hook = lambda h: state.__setitem__("h", h)
    mod.get_axon_ntff_profile_hook = lambda: state["h"]
    sys.modules["antenv.axon_hooks"] = mod
    try:
        from trn_agent_boot.trn_boot import _ntff_profile_via_ctypes
        mod.set_axon_ntff_profile_hook(
            _ntff_profile_via_ctypes("/opt/axon/libaxon_pjrt.so"))
    except Exception:
        pass


def _pack_core(edge_indices, core):
    """Build per-core index tensors.

    Returns:
      agidx [A_SLOTS] int16   phase-A gather idxs (bucket-local src), 0 pad
      bgidx [B_SLOTS] int16   phase-B gather idxs (region staging row), 0 pad
      btv   [B_SLOTS] f32     phase-B window-local target value, -1 pad
    """
    lo = core * TPC
    agidx = np.zeros(A_SLOTS, dtype=np.int16)
    bgidx = np.zeros(B_SLOTS, dtype=np.int16)
    btv = np.full(B_SLOTS, -1.0, dtype=np.float32)

    for r in range(NUM_REL):
        src = np.asarray(edge_indices[r, 0])
        tgt = np.asarray(edge_indices[r, 1])
        m = (tgt >= lo) & (tgt < lo + TPC)
        s = src[m].astype(np.int64)
        t = (tgt[m] - lo).astype(np.int64)
        w = t // WIN
        reg = np.minimum(w // WPR, NREG - 1)
        b = s // BROWS

        # ---- phase A placement: group by (region, bucket) ----
        order_a = np.lexsort((t, b, reg))
        rs, bs = reg[order_a], b[order_a]
        grp = rs * NB + bs
        # rank within group
        change = np.empty(grp.shape[0], dtype=bool)
        if grp.shape[0]:
            change[0] = True
            change[1:] = grp[1:] != grp[:-1]
        starts = np.nonzero(change)[0]
        gid = np.cumsum(change) - 1
        rank = np.arange(grp.shape[0]) - starts[gid]
        counts = np.bincount(grp, minlength=NREG * NB)
        if counts.max(initial=0) > SUBCAP:
            raise RuntimeError(
                f"phase-A overflow r{r} core{core}: {counts.max()}")
        # A-gather g = (r, b, quad); sub-chunk k = reg % 4 at slots
        # [640k, 640k+640) of that gather; global slot offset:
        quad = rs // 4
        sub = rs % 4
        gbase = ((r * NB + bs) * (NREG // 4) + quad) * A_GATHER_SLOTS
        aslot = gbase + sub * SUBCAP + rank
        agidx[aslot] = (s[order_a] - bs * BROWS).astype(np.int16)
        # staging row in region table (partition-major permuted within chunk)
        nblk = SUBCAP // 128
        soff = (r * NB + bs) * SUBCAP
        stage_row_sorted = soff + (rank % 128) * nblk + rank // 128
        stage_row = np.empty(t.shape[0], dtype=np.int64)
        stage_row[order_a] = stage_row_sorted

        # ---- phase B placement: group by (window), target-sorted ----
        order_b = np.lexsort((t,))
        wsB, tsB = w[order_b], t[order_b]
        rowsB = stage_row[order_b]
        changeB = np.empty(wsB.shape[0], dtype=bool)
        if wsB.shape[0]:
            changeB[0] = True
            changeB[1:] = wsB[1:] != wsB[:-1]
        startsB = np.nonzero(changeB)[0]
        gidB = np.cumsum(changeB) - 1
        rankB = np.arange(wsB.shape[0]) - startsB[gidB]
        countsB = np.bincount(wsB, minlength=NWIN)
        if countsB.max(initial=0) > BCAP:
            raise RuntimeError(
                f"phase-B overflow r{r} core{core}: {countsB.max()}")
        pair = wsB // 2
        subw = wsB % 2
        bslot = pair * PAIR_SLOTS + (subw * NUM_REL + r) * BCAP + rankB
        bgidx[bslot] = rowsB.astype(np.int16)
        btv[bslot] = (tsB - wsB * WIN).astype(np.float32)
    return agidx, bgidx, btv


def _block_ranges(packs):
    """Per global 128-slot block: union across cores of the active
    window-local target range [lo, hi).  All-pad blocks get (0, 1)."""
    nblocks = B_SLOTS // 128
    btvs = np.stack([p[2] for p in packs])            # [NCORE, B_SLOTS]
    btvs = btvs.reshape(NCORE, nblocks, 128)
    valid = btvs >= 0
    lob = np.where(valid, btvs, np.inf).min(axis=(0, 2))
    hib = np.where(valid, btvs, -np.inf).max(axis=(0, 2))
    ranges = []
    for j in range(nblocks):
        if np.isfinite(lob[j]):
            ranges.append((int(lob[j]), int(hib[j]) + 1))
        else:
            ranges.append((0, 1))
    return ranges


def _wrap16(idx_flat):
    n = idx_flat.shape[0]
    a = idx_flat.reshape(n // 16, 16).T
    return np.tile(a, (8, 1)).copy()


def _slotmaj(v):
    n = v.shape[0]
    return np.ascontiguousarray(v.reshape(n // 128, 128).T)


def _build_program(ranges):
    import concourse.bacc as bacc
    import concourse.tile as tile
    from concourse import mybir

    nc = bacc.Bacc("TRN2", debug=False, num_swdge_queues=NQUEUE)
    dt = mybir.dt

    xb = [nc.dram_tensor(f"xb{k}", [BROWS, D], dt.bfloat16, kind="ExternalInput")
          for k in range(NB)]
    xto = nc.dram_tensor("xto", [D, TPCP], dt.bfloat16, kind="ExternalInput")
    wt = nc.dram_tensor("wt", [D, 3 * D], dt.bfloat16, kind="ExternalInput")
    agidx_d = nc.dram_tensor("agidx", [128, A_SLOTS // 16], dt.int16,
                             kind="ExternalInput")
    bgidx_d = nc.dram_tensor("bgidx", [128, B_SLOTS // 16], dt.int16,
                             kind="ExternalInput")
    btv_d = nc.dram_tensor("btv", [128, B_SLOTS // 128], dt.float16,
                           kind="ExternalInput")
    iota_d = nc.dram_tensor("iota", [128, WIN], dt.float16, kind="ExternalInput")
    stage = [nc.dram_tensor(f"stage{h}", [STAGE_ROWS, D], dt.bfloat16,
                            kind="ExternalOutput") for h in range(NREG)]
    out_d = nc.dram_tensor("out", [128, TPCP], dt.bfloat16,
                           kind="ExternalOutput")

    nblk_sub = SUBCAP // 128            # 5
    nblk_a = A_GATHER_SLOTS // 128      # 20
    nblk_b = PAIR_SLOTS // 128          # 20

    with tile.TileContext(nc) as tc:
        with (
            tc.tile_pool(name="const", bufs=1) as cpool,
            tc.tile_pool(name="ag", bufs=4) as agpool,
            tc.tile_pool(name="bg", bufs=4) as bgpool,
            tc.tile_pool(name="oh", bufs=12) as ohpool,
            tc.tile_pool(name="gsb", bufs=6) as gsbpool,
            tc.tile_pool(name="xt", bufs=4) as xtpool,
            tc.tile_pool(name="osb", bufs=4) as osbpool,
            tc.tile_pool(name="psA", bufs=4, space="PSUM") as psA,
            tc.tile_pool(name="psB", bufs=3, space="PSUM") as psB,
        ):
            wt_sb = cpool.tile([D, 3 * D], dt.bfloat16)
            nc.sync.dma_start(wt_sb[:], wt[:])
            iota_sb = cpool.tile([128, WIN], dt.float16)
            nc.sync.dma_start(iota_sb[:], iota_d[:])
            agidx_sb = cpool.tile([128, A_SLOTS // 16], dt.int16)
            nc.sync.dma_start(agidx_sb[:], agidx_d[:])
            bgidx_sb = cpool.tile([128, B_SLOTS // 16], dt.int16)
            nc.sync.dma_start(bgidx_sb[:], bgidx_d[:])
            btv_sb = cpool.tile([128, B_SLOTS // 128], dt.float16)
            nc.sync.dma_start(btv_sb[:], btv_d[:])

            qrot = [0]

            def emit_a_one(q, r, b):
                # phase A for regions 4q..4q+3, one (rel, bucket):
                # two gathers of 1280 (ucode handles <= ~2048 idxs per inst)
                gi = ((r * NB + b) * (NREG // 4) + q)
                fb0 = gi * A_GATHER_SLOTS
                soff = (r * NB + b) * SUBCAP
                half = A_GATHER_SLOTS // 2          # 1280
                _kdiag = int(os.environ.get("KDIAG", "0"))
                for hh in range(2):
                    g = agpool.tile([128, nblk_a // 2, D], dt.bfloat16,
                                    tag="ag")
                    fb = fb0 + hh * half
                    if _kdiag:
                        nc.gpsimd.dma_gather(
                            g[:, :_kdiag // 128, :], xb[b][:],
                            agidx_sb[:, fb // 16:(fb + _kdiag) // 16],
                            _kdiag, _kdiag, D,
                            queue_num=0,
                        )
                        qrot[0] += 1
                        continue
                    nc.gpsimd.dma_gather(
                        g[:], xb[b][:],
                        agidx_sb[:, fb // 16:(fb + half) // 16],
                        half, half, D,
                        queue_num=0,
                    )
                    qrot[0] += 1
                    _kaw = int(os.environ.get("KAW", "2"))
                    for k in range(_kaw):
                        h = 4 * q + 2 * hh + k
                        nc.sync.dma_start(
                            stage[h][soff:soff + SUBCAP, :].rearrange(
                                "(p j) o -> p j o", p=128),
                            g[:, k * nblk_sub:(k + 1) * nblk_sub, :],
                        )

            def emit_b_pair(p):
                h = (2 * p) // WPR
                gb = bgpool.tile([128, nblk_b, D], dt.bfloat16, tag="bg")
                boff = p * PAIR_SLOTS
                nc.gpsimd.dma_gather(
                    gb[:], stage[h][:],
                    bgidx_sb[:, boff // 16:(boff + PAIR_SLOTS) // 16],
                    PAIR_SLOTS, PAIR_SLOTS, D,
                    queue_num=0,
                )
                qrot[0] += 1
                for sw in range(2):
                    wv = 2 * p + sw
                    if wv >= NWIN:
                        continue
                    t0 = wv * WIN
                    outp = psB.tile([128, WIN], dt.float32, tag="psB")
                    xt_t = xtpool.tile([D, WIN], dt.bfloat16, tag="xt")
                    nc.sync.dma_start(xt_t[:], xto[:, t0:t0 + WIN])
                    nc.tensor.matmul(
                        outp[:], wt_sb[:, 0:D], xt_t[:],
                        start=True, stop=False,
                    )
                    for r in range(NUM_REL):
                        gps = psA.tile([128, WIN], dt.float32, tag="psA")
                        nc.vector.memset(gps[:], 0.0)
                        jbase = (sw * NUM_REL + r) * nblk_sub
                        for j in range(nblk_sub):
                            bcol = boff // 128 + jbase + j
                            lo, hi = ranges[bcol]
                            oh = ohpool.tile([128, WIN], dt.bfloat16, tag="oh")
                            nc.vector.tensor_tensor(
                                out=oh[:, :hi - lo],
                                in0=btv_sb[:, bcol:bcol + 1]
                                    .to_broadcast([128, hi - lo]),
                                in1=iota_sb[:, lo:hi],
                                op=mybir.AluOpType.is_equal,
                            )
                            nc.tensor.matmul(
                                gps[:, lo:hi],
                                gb[:, jbase + j, :],
                                oh[:, :hi - lo],
                                start=False, stop=(j == nblk_sub - 1),
                                skip_group_check=True,
                            )
                        gsb = gsbpool.tile([128, WIN], dt.bfloat16, tag="gsb")
                        nc.scalar.copy(out=gsb[:], in_=gps[:])
                        nc.tensor.matmul(
                            outp[:], wt_sb[:, (1 + r) * D:(2 + r) * D], gsb[:],
                            start=False, stop=(r == NUM_REL - 1),
                        )
                    osb = osbpool.tile([128, WIN], dt.bfloat16, tag="osb")
                    nc.scalar.copy(out=osb[:], in_=outp[:])
                    nc.sync.dma_start(out_d[:, t0:t0 + WIN], osb[:])

            # pipeline: quad-0 phase A first; quad-1 phase-A gathers
            # interleaved 1:1 with quad-0's B pairs (Pool engine is in-order,
            # so this keeps phase-A DMA flowing while B consumes quad 0).
            _phase = os.environ.get("KPHASE", "")
            _maxpair = int(os.environ.get("KMAXPAIR", str(NPAIR)))
            _maxa = int(os.environ.get("KMAXA", "64"))
            _acnt = [0]
            rb = [(r, b) for r in range(NUM_REL) for b in range(NB)]
            for (r, b) in rb:
                if _acnt[0] < _maxa:
                    emit_a_one(0, r, b)
                    _acnt[0] += 1
            ppq = 32            # pairs per quad (4 regions x 16 win / 2)
            for p in range(ppq):
                if _phase != "A" and p < _maxpair:
                    emit_b_pair(p)
                if _acnt[0] < _maxa:
                    emit_a_one(1, *rb[p])
                    _acnt[0] += 1
            for p in range(ppq, NPAIR):
                if _phase != "A" and p < _maxpair:
                    emit_b_pair(p)
    nc.compile()
    return nc


_NC_CACHE = {}


def kernel(x, W0, W1, W_self, edge_indices):
    import ml_dtypes
    from concourse import bass_utils
    from concourse.bass_utils import run_bass_kernel_spmd

    _register_profile_hook()
    bass_utils.upload_artifacts = lambda tmpdir: "local://" + tmpdir

    x = np.asarray(x)
    W0 = np.asarray(W0)
    W1 = np.asarray(W1)
    W_self = np.asarray(W_self)
    edge_indices = np.asarray(edge_indices)

    bf16 = ml_dtypes.bfloat16
    x16 = x.astype(bf16)
    xbufs = [np.ascontiguousarray(x16[k * BROWS:(k + 1) * BROWS])
             for k in range(NB)]
    wt = np.concatenate([W_self.T, W0.T, W1.T], axis=1).astype(bf16)
    iota = np.tile(np.arange(WIN, dtype=np.float16), (128, 1))

    packs = [_pack_core(edge_indices, c) for c in range(NCORE)]
    ranges = _block_ranges(packs)
    if "nc" not in _NC_CACHE:
        _NC_CACHE["nc"] = _build_program(ranges)
    nc = _NC_CACHE["nc"]

    in_maps = []
    for c in range(NCORE):
        agidx, bgidx, btv = packs[c]
        im = {f"xb{k}": xbufs[k] for k in range(NB)}
        xt = np.zeros((D, TPCP), dtype=bf16)
        xt[:, :TPC] = x16[c * TPC:(c + 1) * TPC].T
        im["xto"] = xt
        im["wt"] = wt
        im["agidx"] = _wrap16(agidx)
        im["bgidx"] = _wrap16(bgidx)
        im["btv"] = _slotmaj(btv.astype(np.float16))
        im["iota"] = iota
        in_maps.append(im)

    trace = os.environ.get("KBENCH_TRACE", "0") == "1"
    res = run_bass_kernel_spmd(nc, in_maps, core_ids=list(range(NCORE)),
                               trace=trace)
    if trace:
        print("HW exec time:", res.exec_time_ns, "ns")
        _NC_CACHE["exec_time_ns"] = res.exec_time_ns

    out = np.empty((N, D), dtype=np.float32)
    for c in range(NCORE):
        o = np.asarray(res.results[c]["out"])          # [128, TPCP] bf16
        out[c * TPC:(c + 1) * TPC] = o[:, :TPC].T.astype(np.float32)
    return out
